# revision 4
# baseline (speedup 1.0000x reference)
"""Trainium2 Bass kernel for nn_ModalGenerator (MoE-routed cross-modal generator).

Strategy:
  - seq_len==1 => softmax over one key is identically 1, so attention output is
    just v = tgt @ wv.T + bv. Fold wv/ao_w into one 512x512 matrix per layer
    (host-side), and fold (1-rw) into the output projection.
  - MoE routing: only columns with missing_type==1 need generator 0 (img->text)
    and only missing_type==2 need generator 1 (text->img). Host gathers those
    columns, the device runs the generators on the compacted columns only
    (~1/4 of the batch each), host scatters results back. missing_type==3 rows
    use the (host-computed, tiny) prior MLP; other rows pass through.
  - Data-parallel over 8 NeuronCores: columns sharded, weights replicated.
  - Activations live transposed [H(partitions), cols(free)]. All matmuls in
    float32r (e8m11) at full PE rate. LayerNorm stats via ones-matmul (column
    sums, already broadcast across partitions), rsqrt via bit-hack + Newton on
    VectorE, exact Gelu on ScalarE (single ACT table set for the whole kernel).
"""

import math

import numpy as np

import concourse.bacc as bacc
import concourse.mybir as mybir
import concourse.tile as tile
from concourse.bass_utils import run_bass_kernel_spmd

f32 = mybir.dt.float32
f32r = mybir.dt.float32r
i32 = mybir.dt.int32
AF = mybir.ActivationFunctionType
ALU = mybir.AluOpType

H = 512
L = 3
N_CORES = 8
KC = H // 128            # 4 K-chunks of the hidden dim
FH = 4 * H               # 2048 FFN hidden
FKC = FH // 128          # 16
LN_EPS = 1e-5
MAGIC = 0x5F3759DF

# param pack column layout (per generator): [128, 128] f32
_P_IPB = 0
_P_LAYER = 4             # + 40*l: ba 0..3 | f1b 4..19 | f2b 20..23
#                                 | ln1g 24..27 | ln1b 28..31 | ln2g 32..35 | ln2b 36..39
_P_OPB = 124


def _round_f32r(a):
    """Round-to-nearest-even fp32 -> fp32r (e8m11: low 12 mantissa bits zero)."""
    b = np.ascontiguousarray(a, dtype=np.float32).view(np.uint32).copy()
    b += np.uint32(0x7FF) + ((b >> np.uint32(12)) & np.uint32(1))
    b &= np.uint32(0xFFFFF000)
    return b.view(np.float32)


def _pack_pcol(vec):
    """[n*128] vector -> [128, n] chunk-column layout."""
    return np.ascontiguousarray(np.asarray(vec, np.float32).reshape(-1, 128).T)


def _sb_pack(wT):
    """[K, M] (K mult of 128) -> [128, (K/128)*M] SBUF chunk-major layout."""
    K, M = wT.shape
    return np.ascontiguousarray(
        wT.reshape(K // 128, 128, M).transpose(1, 0, 2).reshape(128, -1))


def _ntiles(C):
    if C <= 512:
        return [(0, C)]
    h = ((C // 2) + 31) // 32 * 32
    return [(0, h), (h, C)]


def _build_program(C0, C1, skip_b):
    nc = bacc.Bacc("TRN2", target_bir_lowering=False, debug=False,
                   num_devices=N_CORES)

    dram = {}
    for g, C in ((0, C0), (1, C1)):
        dram[f"src{g}"] = nc.dram_tensor(f"src{g}", [128, KC * C], f32r, kind="ExternalInput")
        dram[f"tgt{g}"] = nc.dram_tensor(f"tgt{g}", [128, KC * C], f32r, kind="ExternalInput")
        dram[f"io{g}"] = nc.dram_tensor(f"io{g}", [128, 2 * KC * H], f32r, kind="ExternalInput")
        dram[f"wa{g}"] = nc.dram_tensor(f"wa{g}", [L, 128, KC * H], f32r, kind="ExternalInput")
        dram[f"f1{g}"] = nc.dram_tensor(f"f1{g}", [L, 128, KC * FH], f32r, kind="ExternalInput")
        dram[f"f2{g}"] = nc.dram_tensor(f"f2{g}", [L, 128, FKC * H], f32r, kind="ExternalInput")
        dram[f"par{g}"] = nc.dram_tensor(f"par{g}", [128, 128], f32, kind="ExternalInput")
        dram[f"out{g}"] = nc.dram_tensor(f"out{g}", [128, KC * C], f32, kind="ExternalOutput")
    dram["ones"] = nc.dram_tensor("ones", [128, 128], f32r, kind="ExternalInput")

    h_bufs = 2 if max(C0, C1) <= 512 else 1

    with tile.TileContext(nc) as tc:
        with (
            tc.tile_pool(name="sb", bufs=2) as sb,
            tc.tile_pool(name="ps", bufs=4, space="PSUM") as psp,
        ):
            ones = sb.tile([128, 128], f32r, tag="ones", bufs=1)
            nc.sync.dma_start(ones[:], dram["ones"].ap())

            def ln_stats(y, cs):
                """column sums of y and y^2 (broadcast over partitions) -> rstd, m."""
                c0, c1 = cs
                Ct = c1 - c0
                s_ps = psp.tile([128, Ct], f32, tag="s", bufs=2)
                q_ps = psp.tile([128, Ct], f32, tag="q", bufs=2)
                for k in range(KC):
                    nc.tensor.matmul(s_ps[:], ones[:], y[:, k, c0:c1],
                                     start=(k == 0), stop=(k == KC - 1))
                for k in range(KC):
                    ysq = sb.tile([128, Ct], f32r, tag="ysq")
                    nc.scalar.activation(ysq[:], y[:, k, c0:c1], AF.Square)
                    nc.tensor.matmul(q_ps[:], ones[:], ysq[:],
                                     start=(k == 0), stop=(k == KC - 1))
                m_bc = sb.tile([128, Ct], f32, tag="m")
                nc.vector.tensor_scalar(m_bc[:], s_ps[:], 1.0 / H, None, ALU.mult)
                msq = sb.tile([128, Ct], f32, tag="msq", bufs=1)
                nc.vector.tensor_mul(msq[:], m_bc[:], m_bc[:])
                z = sb.tile([128, Ct], f32, tag="z")
                nc.vector.scalar_tensor_tensor(z[:], q_ps[:], 1.0 / H, msq[:],
                                               ALU.mult, ALU.subtract)
                nc.vector.tensor_scalar(z[:], z[:], LN_EPS, None, ALU.add)
                ti = sb.tile([128, Ct], i32, tag="ti", bufs=1)
                nc.vector.tensor_scalar(ti[:], z[:].bitcast(i32), 1, None,
                                        ALU.arith_shift_right)
                rstd = sb.tile([128, Ct], f32, tag="rstd", bufs=2)
                nc.vector.tensor_scalar(rstd[:].bitcast(i32), ti[:], -1, MAGIC,
                                        ALU.mult, ALU.add)
                for _ in range(3):
                    u = sb.tile([128, Ct], f32, tag="u")
                    nc.vector.tensor_mul(u[:], rstd[:], rstd[:])
                    w = sb.tile([128, Ct], f32, tag="w")
                    nc.vector.scalar_tensor_tensor(w[:], u[:], -0.5, z[:],
                                                   ALU.mult, ALU.mult)
                    rstd2 = sb.tile([128, Ct], f32, tag="rstd", bufs=2)
                    nc.vector.scalar_tensor_tensor(rstd2[:], w[:], 1.5, rstd[:],
                                                   ALU.add, ALU.mult)
                    rstd = rstd2
                return m_bc, rstd

            def ln_apply(y, xn, cs, par, gcol, bcol, m_bc, rstd, skip_beta):
                c0, c1 = cs
                for m in range(KC):
                    u1 = sb.tile([128, c1 - c0], f32, tag="u1")
                    nc.vector.tensor_sub(u1[:], y[:, m, c0:c1], m_bc[:])
                    if skip_beta:
                        nc.vector.scalar_tensor_tensor(
                            xn[:, m, c0:c1], u1[:], par[:, gcol + m:gcol + m + 1],
                            rstd[:], ALU.mult, ALU.mult)
                    else:
                        u2 = sb.tile([128, c1 - c0], f32, tag="u2", bufs=1)
                        nc.vector.scalar_tensor_tensor(
                            u2[:], u1[:], par[:, gcol + m:gcol + m + 1],
                            rstd[:], ALU.mult, ALU.mult)
                        nc.vector.tensor_scalar(
                            xn[:, m, c0:c1], u2[:], par[:, bcol + m:bcol + m + 1],
                            None, ALU.add)

            for g, C in ((0, C0), (1, C1)):
                tiles = _ntiles(C)
                src = sb.tile([128, KC, C], f32r, tag="src", bufs=1)
                tgt = sb.tile([128, KC, C], f32r, tag="tgt", bufs=1)
                nc.sync.dma_start(src[:], dram[f"src{g}"].ap())
                nc.sync.dma_start(tgt[:], dram[f"tgt{g}"].ap())
                par = sb.tile([128, 128], f32, tag="par")
                nc.sync.dma_start(par[:], dram[f"par{g}"].ap())
                iow = sb.tile([128, KC * H], f32r, tag="io", bufs=1)
                nc.sync.dma_start(iow[:], dram[f"io{g}"].ap()[:, 0:KC * H])

                # input proj: x = ipwT.T @ src + ipb
                x = sb.tile([128, KC, C], f32r, tag="x", bufs=2)
                for cs in tiles:
                    for m in range(KC):
                        ps = psp.tile([128, cs[1] - cs[0]], f32, tag="mm")
                        for k in range(KC):
                            nc.tensor.matmul(
                                ps[:], iow[:, k * H + 128 * m:k * H + 128 * (m + 1)],
                                src[:, k, cs[0]:cs[1]],
                                start=(k == 0), stop=(k == KC - 1))
                        nc.vector.tensor_scalar(
                            x[:, m, cs[0]:cs[1]], ps[:],
                            par[:, _P_IPB + m:_P_IPB + m + 1], None, ALU.add)

                for l in range(L):
                    pb = _P_LAYER + 40 * l
                    wa = sb.tile([128, KC * H], f32r, tag="wa", bufs=1)
                    nc.sync.dma_start(wa[:], dram[f"wa{g}"].ap()[l])
                    f1w = sb.tile([128, KC * FH], f32r, tag="f1", bufs=1)
                    nc.sync.dma_start(f1w[:], dram[f"f1{g}"].ap()[l])
                    f2w = sb.tile([128, FKC, H], f32r, tag="f2", bufs=1)
                    nc.sync.dma_start(f2w[:], dram[f"f2{g}"].ap()[l])

                    # ---- attention(=v proj) + residual + LN1 ----
                    xn = sb.tile([128, KC, C], f32r, tag="x", bufs=2)
                    for cs in tiles:
                        y = sb.tile([128, KC, C], f32r, tag="y")
                        for m in range(KC):
                            ps = psp.tile([128, cs[1] - cs[0]], f32, tag="mm")
                            for k in range(KC):
                                nc.tensor.matmul(
                                    ps[:], wa[:, k * H + 128 * m:k * H + 128 * (m + 1)],
                                    tgt[:, k, cs[0]:cs[1]],
                                    start=(k == 0), stop=(k == KC - 1))
                            nc.vector.scalar_tensor_tensor(
                                y[:, m, cs[0]:cs[1]], ps[:],
                                par[:, pb + m:pb + m + 1], x[:, m, cs[0]:cs[1]],
                                ALU.add, ALU.add)
                        m_bc, rstd = ln_stats(y, cs)
                        ln_apply(y, xn, cs, par, pb + 24, pb + 28, m_bc, rstd,
                                 skip_b[g][0])
                    x = xn

                    # ---- FFN + residual + LN2 ----
                    xn2 = sb.tile([128, KC, C], f32r, tag="x", bufs=2)
                    for cs in tiles:
                        Ct = cs[1] - cs[0]
                        hh = sb.tile([128, FKC, Ct], f32r, tag="h", bufs=h_bufs)
                        for m in range(FKC):
                            ps = psp.tile([128, Ct], f32, tag="mm")
                            for k in range(KC):
                                nc.tensor.matmul(
                                    ps[:], f1w[:, k * FH + 128 * m:k * FH + 128 * (m + 1)],
                                    x[:, k, cs[0]:cs[1]],
                                    start=(k == 0), stop=(k == KC - 1))
                            nc.scalar.activation(hh[:, m, :], ps[:], AF.Gelu,
                                                 bias=par[:, pb + 4 + m:pb + 4 + m + 1])
                        y2 = sb.tile([128, KC, C], f32r, tag="y")
                        for m in range(KC):
                            ps = psp.tile([128, Ct], f32, tag="mm")
                            for k in range(FKC):
                                nc.tensor.matmul(
                                    ps[:], f2w[:, k, 128 * m:128 * (m + 1)], hh[:, k, :],
                                    start=(k == 0), stop=(k == FKC - 1))
                            nc.vector.scalar_tensor_tensor(
                                y2[:, m, cs[0]:cs[1]], ps[:],
                                par[:, pb + 20 + m:pb + 20 + m + 1],
                                x[:, m, cs[0]:cs[1]], ALU.add, ALU.add)
                        m_bc, rstd = ln_stats(y2, cs)
                        ln_apply(y2, xn2, cs, par, pb + 32, pb + 36, m_bc, rstd,
                                 skip_b[g][1])
                    x = xn2

                # output proj (pre-scaled by (1-rw)); host adds rw*tgt
                opw = sb.tile([128, KC * H], f32r, tag="io", bufs=1)
                nc.sync.dma_start(opw[:], dram[f"io{g}"].ap()[:, KC * H:2 * KC * H])
                for cs in tiles:
                    for m in range(KC):
                        ps = psp.tile([128, cs[1] - cs[0]], f32, tag="mm")
                        for k in range(KC):
                            nc.tensor.matmul(
                                ps[:],
                                opw[:, k * H + 128 * m:k * H + 128 * (m + 1)],
                                x[:, k, cs[0]:cs[1]],
                                start=(k == 0), stop=(k == KC - 1))
                        ot = sb.tile([128, cs[1] - cs[0]], f32, tag="o")
                        nc.vector.tensor_scalar(
                            ot[:], ps[:], par[:, _P_OPB + m:_P_OPB + m + 1],
                            None, ALU.add)
                        nc.sync.dma_start(
                            dram[f"out{g}"].ap()[:, m * C + cs[0]:m * C + cs[1]], ot[:])

    nc.compile()
    return nc


_CACHE = {}


def _get_program(C0, C1, skip_b):
    key = (C0, C1, skip_b)
    if key not in _CACHE:
        _CACHE[key] = _build_program(C0, C1, skip_b)
    return _CACHE[key]


def _prep_gen_weights(i, g_ipw, g_ipb, g_qkv_w, g_qkv_b, g_ao_w, g_ao_b,
                      g_ln1g, g_ln1b, g_ln2g, g_ln2b, g_f1w, g_f1b, g_f2w,
                      g_f2b, g_opw, g_opb, g_rw):
    wa, ba = [], []
    for l in range(L):
        _wq, _wk, wv = np.split(g_qkv_w[i, l], 3, axis=0)
        _bq, _bk, bv = np.split(g_qkv_b[i, l], 3)
        wa.append((g_ao_w[i, l] @ wv).T)                 # [K=H, M=H]
        ba.append(g_ao_b[i, l] + bv @ g_ao_w[i, l].T)
    rw = float(g_rw[i])
    io = np.concatenate([_sb_pack(_round_f32r(g_ipw[i].T)),
                         _sb_pack(_round_f32r((1.0 - rw) * g_opw[i].T))], axis=1)
    waP = np.stack([_sb_pack(_round_f32r(wa[l])) for l in range(L)])
    f1P = np.stack([_sb_pack(_round_f32r(g_f1w[i, l].T)) for l in range(L)])
    f2P = np.stack([_sb_pack(_round_f32r(g_f2w[i, l].T)) for l in range(L)])

    par = np.zeros((128, 128), np.float32)
    par[:, _P_IPB:_P_IPB + KC] = _pack_pcol(g_ipb[i])
    for l in range(L):
        pb = _P_LAYER + 40 * l
        par[:, pb:pb + 4] = _pack_pcol(ba[l])
        par[:, pb + 4:pb + 20] = _pack_pcol(g_f1b[i, l])
        par[:, pb + 20:pb + 24] = _pack_pcol(g_f2b[i, l])
        par[:, pb + 24:pb + 28] = _pack_pcol(g_ln1g[i, l])
        par[:, pb + 28:pb + 32] = _pack_pcol(g_ln1b[i, l])
        par[:, pb + 32:pb + 36] = _pack_pcol(g_ln2g[i, l])
        par[:, pb + 36:pb + 40] = _pack_pcol(g_ln2b[i, l])
    par[:, _P_OPB:_P_OPB + KC] = _pack_pcol((1.0 - rw) * g_opb[i])

    skip = (bool(np.all(g_ln1b[i] == 0.0)), bool(np.all(g_ln2b[i] == 0.0)))
    return {"io": io, "wa": waP, "f1": f1P, "f2": f2P, "par": par}, skip, rw


def _prepare(inputs):
    """Host-side prep. Returns (nc, in_maps, assemble) where assemble(results)
    builds the final outputs."""
    image = np.asarray(inputs["image_features"], np.float32)
    text = np.asarray(inputs["text_features"], np.float32)
    mt = np.asarray(inputs["missing_type"])

    idx1 = np.nonzero(mt == 1)[0]      # gen0 (img -> text) fills text
    idx2 = np.nonzero(mt == 2)[0]      # gen1 (text -> img) fills img
    idx3 = np.nonzero(mt == 3)[0]

    gw = {k: np.asarray(v) for k, v in inputs.items() if k.startswith("g_")}
    w0, skip0, rw0 = _prep_gen_weights(0, **gw)
    w1, skip1, rw1 = _prep_gen_weights(1, **gw)

    # prior MLP on host (tiny)
    pe = np.asarray(inputs["prior_emb"], np.float64)
    t = pe @ np.asarray(inputs["prior_w1"], np.float64).T + np.asarray(inputs["prior_b1"], np.float64)
    t = 0.5 * t * (1.0 + np.vectorize(math.erf)(t / math.sqrt(2.0)))
    prior = (t @ np.asarray(inputs["prior_w2"], np.float64).T
             + np.asarray(inputs["prior_b2"], np.float64)).astype(np.float32)
    p_img, p_text = prior[0, :H], prior[0, H:]

    imgT = np.ascontiguousarray(image.T)
    textT = np.ascontiguousarray(text.T)

    def shard_cols(Tsrc, Ttgt, idx):
        n_pc = max(1, -(-len(idx) // N_CORES))
        C = max(256, -(-n_pc // 64) * 64)
        pad = np.zeros(N_CORES * C, np.int64)
        pad[:len(idx)] = idx
        pad = pad.reshape(N_CORES, C)
        return C, [_sb_pack(_round_f32r(Tsrc[:, pad[c]])) for c in range(N_CORES)], \
            [_sb_pack(_round_f32r(Ttgt[:, pad[c]])) for c in range(N_CORES)]

    C0, src0, tgt0 = shard_cols(imgT, textT, idx1)
    C1, src1, tgt1 = shard_cols(textT, imgT, idx2)

    nc = _get_program(C0, C1, (skip0, skip1))

    ones = np.ones((128, 128), np.float32)
    in_maps = []
    for c in range(N_CORES):
        in_maps.append({
            "src0": src0[c], "tgt0": tgt0[c], "src1": src1[c], "tgt1": tgt1[c],
            "io0": w0["io"], "wa0": w0["wa"], "f10": w0["f1"], "f20": w0["f2"],
            "par0": w0["par"],
            "io1": w1["io"], "wa1": w1["wa"], "f11": w1["f1"], "f21": w1["f2"],
            "par1": w1["par"],
            "ones": ones,
        })

    def assemble(results):
        def gather_out(name, C, idx, rw, full):
            cols = [results[c][name].reshape(128, KC, C).transpose(1, 0, 2).reshape(H, C)
                    for c in range(N_CORES)]
            allc = np.concatenate(cols, axis=1)[:, :len(idx)]
            return rw * full[idx] + allc.T

        enhanced_text = text.copy()
        if len(idx1):
            enhanced_text[idx1] = gather_out("out0", C0, idx1, rw0, text)
        enhanced_img = image.copy()
        if len(idx2):
            enhanced_img[idx2] = gather_out("out1", C1, idx2, rw1, image)
        if len(idx3):
            enhanced_img[idx3] = p_img
            enhanced_text[idx3] = p_text
        return enhanced_img, enhanced_text

    return nc, in_maps, assemble


def kernel(**inputs):
    nc, in_maps, assemble = _prepare(inputs)
    res = run_bass_kernel_spmd(nc, in_maps, list(range(N_CORES)))
    return assemble(res.results)


# revision 5
# speedup vs baseline: 31.3402x; 31.3402x over previous
"""Trainium2 Bass kernel for nn_ModalGenerator (MoE-routed cross-modal generator).

Strategy:
  - seq_len==1 => softmax over one key is identically 1, so attention output is
    just v = tgt @ wv.T + bv. Fold wv/ao_w into one 512x512 matrix per layer
    (host-side), and fold (1-rw) into the output projection.
  - MoE routing: only columns with missing_type==1 need generator 0 (img->text)
    and only missing_type==2 need generator 1 (text->img). Host gathers those
    columns, the device runs the generators on the compacted columns only
    (~1/4 of the batch each), host scatters results back. missing_type==3 rows
    use the (host-computed, tiny) prior MLP; other rows pass through.
  - Data-parallel over 8 NeuronCores: columns sharded, weights replicated.
  - Activations live transposed [H(partitions), cols(free)]. All matmuls in
    float32r (e8m11) at full PE rate. LayerNorm stats via ones-matmul (column
    sums, already broadcast across partitions), rsqrt via bit-hack + Newton on
    VectorE, exact Gelu on ScalarE (single ACT table set for the whole kernel).
"""

import math

import numpy as np

import concourse.bacc as bacc
import concourse.mybir as mybir
import concourse.tile as tile
from concourse.bass_utils import run_bass_kernel_spmd

f32 = mybir.dt.float32
f32r = mybir.dt.float32r
i32 = mybir.dt.int32
AF = mybir.ActivationFunctionType
ALU = mybir.AluOpType

H = 512
L = 3
N_CORES = 8
KC = H // 128            # 4 K-chunks of the hidden dim
FH = 4 * H               # 2048 FFN hidden
FKC = FH // 128          # 16
LN_EPS = 1e-5
MAGIC = 0x5F3759DF

# param pack column layout (per generator): [128, 128] f32
_P_IPB = 0
_P_LAYER = 4             # + 40*l: ba 0..3 | f1b 4..19 | f2b 20..23
#                                 | ln1g 24..27 | ln1b 28..31 | ln2g 32..35 | ln2b 36..39
_P_OPB = 124


def _round_f32r(a):
    """Round-to-nearest-even fp32 -> fp32r (e8m11: low 12 mantissa bits zero)."""
    b = np.ascontiguousarray(a, dtype=np.float32).view(np.uint32).copy()
    b += np.uint32(0x7FF) + ((b >> np.uint32(12)) & np.uint32(1))
    b &= np.uint32(0xFFFFF000)
    return b.view(np.float32)


def _pack_pcol(vec):
    """[n*128] vector -> [128, n] chunk-column layout."""
    return np.ascontiguousarray(np.asarray(vec, np.float32).reshape(-1, 128).T)


def _sb_pack(wT):
    """[K, M] (K mult of 128) -> [128, (K/128)*M] SBUF chunk-major layout."""
    K, M = wT.shape
    return np.ascontiguousarray(
        wT.reshape(K // 128, 128, M).transpose(1, 0, 2).reshape(128, -1))


def _ntiles(C):
    if C <= 512:
        return [(0, C)]
    h = ((C // 2) + 31) // 32 * 32
    return [(0, h), (h, C)]


def _build_program(C0, C1, skip_b):
    nc = bacc.Bacc("TRN2", target_bir_lowering=False, debug=False,
                   num_devices=N_CORES)

    dram = {}
    for g, C in ((0, C0), (1, C1)):
        dram[f"src{g}"] = nc.dram_tensor(f"src{g}", [128, KC * C], f32r, kind="ExternalInput")
        dram[f"tgt{g}"] = nc.dram_tensor(f"tgt{g}", [128, KC * C], f32r, kind="ExternalInput")
        dram[f"io{g}"] = nc.dram_tensor(f"io{g}", [128, 2 * KC * H], f32r, kind="ExternalInput")
        dram[f"wa{g}"] = nc.dram_tensor(f"wa{g}", [L, 128, KC * H], f32r, kind="ExternalInput")
        dram[f"f1{g}"] = nc.dram_tensor(f"f1{g}", [L, 128, KC * FH], f32r, kind="ExternalInput")
        dram[f"f2{g}"] = nc.dram_tensor(f"f2{g}", [L, 128, FKC * H], f32r, kind="ExternalInput")
        dram[f"par{g}"] = nc.dram_tensor(f"par{g}", [128, 128], f32, kind="ExternalInput")
        dram[f"out{g}"] = nc.dram_tensor(f"out{g}", [128, KC * C], f32, kind="ExternalOutput")
    dram["ones"] = nc.dram_tensor("ones", [128, 128], f32r, kind="ExternalInput")

    h_bufs = 2 if max(C0, C1) <= 512 else 1

    with tile.TileContext(nc) as tc:
        with (
            tc.tile_pool(name="sb", bufs=2) as sb,
            tc.tile_pool(name="ps", bufs=4, space="PSUM") as psp,
        ):
            ones = sb.tile([128, 128], f32r, tag="ones", bufs=1)
            nc.sync.dma_start(ones[:], dram["ones"].ap())

            def ln_stats(y, cs):
                """column sums of y and y^2 (broadcast over partitions) -> rstd, m."""
                c0, c1 = cs
                Ct = c1 - c0
                s_ps = psp.tile([128, Ct], f32, tag="s", bufs=2)
                q_ps = psp.tile([128, Ct], f32, tag="q", bufs=2)
                for k in range(KC):
                    nc.tensor.matmul(s_ps[:], ones[:], y[:, k, c0:c1],
                                     start=(k == 0), stop=(k == KC - 1))
                for k in range(KC):
                    ysq = sb.tile([128, Ct], f32r, tag="ysq")
                    nc.scalar.activation(ysq[:], y[:, k, c0:c1], AF.Square)
                    nc.tensor.matmul(q_ps[:], ones[:], ysq[:],
                                     start=(k == 0), stop=(k == KC - 1))
                m_bc = sb.tile([128, Ct], f32, tag="m")
                nc.vector.tensor_scalar(m_bc[:], s_ps[:], 1.0 / H, None, ALU.mult)
                msq = sb.tile([128, Ct], f32, tag="msq", bufs=1)
                nc.vector.tensor_mul(msq[:], m_bc[:], m_bc[:])
                z = sb.tile([128, Ct], f32, tag="z")
                nc.vector.scalar_tensor_tensor(z[:], q_ps[:], 1.0 / H, msq[:],
                                               ALU.mult, ALU.subtract)
                nc.vector.tensor_scalar(z[:], z[:], LN_EPS, None, ALU.add)
                ti = sb.tile([128, Ct], i32, tag="ti", bufs=1)
                nc.vector.tensor_scalar(ti[:], z[:].bitcast(i32), 1, None,
                                        ALU.arith_shift_right)
                rstd = sb.tile([128, Ct], f32, tag="rstd", bufs=2)
                nc.vector.tensor_scalar(rstd[:].bitcast(i32), ti[:], -1, MAGIC,
                                        ALU.mult, ALU.add)
                for _ in range(2):
                    u = sb.tile([128, Ct], f32, tag="u", bufs=1)
                    nc.vector.tensor_mul(u[:], rstd[:], rstd[:])
                    w = sb.tile([128, Ct], f32, tag="w", bufs=1)
                    nc.vector.scalar_tensor_tensor(w[:], u[:], -0.5, z[:],
                                                   ALU.mult, ALU.mult)
                    rstd2 = sb.tile([128, Ct], f32, tag="rstd", bufs=2)
                    nc.vector.scalar_tensor_tensor(rstd2[:], w[:], 1.5, rstd[:],
                                                   ALU.add, ALU.mult)
                    rstd = rstd2
                return m_bc, rstd

            def ln_apply(y, xn, cs, par, gcol, bcol, m_bc, rstd, skip_beta):
                c0, c1 = cs
                for m in range(KC):
                    u1 = sb.tile([128, c1 - c0], f32, tag="u1")
                    nc.vector.tensor_sub(u1[:], y[:, m, c0:c1], m_bc[:])
                    if skip_beta:
                        nc.vector.scalar_tensor_tensor(
                            xn[:, m, c0:c1], u1[:], par[:, gcol + m:gcol + m + 1],
                            rstd[:], ALU.mult, ALU.mult)
                    else:
                        u2 = sb.tile([128, c1 - c0], f32, tag="u2", bufs=1)
                        nc.vector.scalar_tensor_tensor(
                            u2[:], u1[:], par[:, gcol + m:gcol + m + 1],
                            rstd[:], ALU.mult, ALU.mult)
                        nc.vector.tensor_scalar(
                            xn[:, m, c0:c1], u2[:], par[:, bcol + m:bcol + m + 1],
                            None, ALU.add)

            for g, C in ((0, C0), (1, C1)):
                tiles = _ntiles(C)
                src = sb.tile([128, KC, C], f32r, tag="x", bufs=2)
                tgt = sb.tile([128, KC, C], f32r, tag="tgt", bufs=1)
                nc.sync.dma_start(src[:], dram[f"src{g}"].ap())
                nc.sync.dma_start(tgt[:], dram[f"tgt{g}"].ap())
                par = sb.tile([128, 128], f32, tag="par", bufs=1)
                nc.sync.dma_start(par[:], dram[f"par{g}"].ap())
                iow = sb.tile([128, KC * H], f32r, tag="io", bufs=1)
                nc.sync.dma_start(iow[:], dram[f"io{g}"].ap()[:, 0:KC * H])

                # input proj: x = ipwT.T @ src + ipb
                x = sb.tile([128, KC, C], f32r, tag="x", bufs=2)
                for cs in tiles:
                    for m in range(KC):
                        ps = psp.tile([128, cs[1] - cs[0]], f32, tag="mm")
                        for k in range(KC):
                            nc.tensor.matmul(
                                ps[:], iow[:, k * H + 128 * m:k * H + 128 * (m + 1)],
                                src[:, k, cs[0]:cs[1]],
                                start=(k == 0), stop=(k == KC - 1))
                        nc.vector.tensor_scalar(
                            x[:, m, cs[0]:cs[1]], ps[:],
                            par[:, _P_IPB + m:_P_IPB + m + 1], None, ALU.add)

                for l in range(L):
                    pb = _P_LAYER + 40 * l
                    wa = sb.tile([128, KC * H], f32r, tag="wa", bufs=2)
                    nc.sync.dma_start(wa[:], dram[f"wa{g}"].ap()[l])
                    f1w = sb.tile([128, KC * FH], f32r, tag="f1", bufs=1)
                    nc.sync.dma_start(f1w[:], dram[f"f1{g}"].ap()[l])
                    f2w = sb.tile([128, FKC, H], f32r, tag="f2", bufs=1)
                    nc.sync.dma_start(f2w[:], dram[f"f2{g}"].ap()[l])

                    # ---- attention(=v proj) + residual + LN1 ----
                    xn = sb.tile([128, KC, C], f32r, tag="x", bufs=2)
                    for cs in tiles:
                        y = sb.tile([128, KC, C], f32r, tag="y")
                        for m in range(KC):
                            ps = psp.tile([128, cs[1] - cs[0]], f32, tag="mm")
                            for k in range(KC):
                                nc.tensor.matmul(
                                    ps[:], wa[:, k * H + 128 * m:k * H + 128 * (m + 1)],
                                    tgt[:, k, cs[0]:cs[1]],
                                    start=(k == 0), stop=(k == KC - 1))
                            nc.vector.scalar_tensor_tensor(
                                y[:, m, cs[0]:cs[1]], ps[:],
                                par[:, pb + m:pb + m + 1], x[:, m, cs[0]:cs[1]],
                                ALU.add, ALU.add)
                        m_bc, rstd = ln_stats(y, cs)
                        ln_apply(y, xn, cs, par, pb + 24, pb + 28, m_bc, rstd,
                                 skip_b[g][0])
                    x = xn

                    # ---- FFN + residual + LN2 ----
                    xn2 = sb.tile([128, KC, C], f32r, tag="x", bufs=2)
                    for cs in tiles:
                        Ct = cs[1] - cs[0]
                        hh = sb.tile([128, FKC, Ct], f32r, tag="h", bufs=h_bufs)
                        for m in range(FKC):
                            ps = psp.tile([128, Ct], f32, tag="mm")
                            for k in range(KC):
                                nc.tensor.matmul(
                                    ps[:], f1w[:, k * FH + 128 * m:k * FH + 128 * (m + 1)],
                                    x[:, k, cs[0]:cs[1]],
                                    start=(k == 0), stop=(k == KC - 1))
                            nc.scalar.activation(hh[:, m, :], ps[:], AF.Gelu,
                                                 bias=par[:, pb + 4 + m:pb + 4 + m + 1])
                        y2 = sb.tile([128, KC, C], f32r, tag="y")
                        for m in range(KC):
                            ps = psp.tile([128, Ct], f32, tag="mm")
                            for k in range(FKC):
                                nc.tensor.matmul(
                                    ps[:], f2w[:, k, 128 * m:128 * (m + 1)], hh[:, k, :],
                                    start=(k == 0), stop=(k == FKC - 1))
                            nc.vector.scalar_tensor_tensor(
                                y2[:, m, cs[0]:cs[1]], ps[:],
                                par[:, pb + 20 + m:pb + 20 + m + 1],
                                x[:, m, cs[0]:cs[1]], ALU.add, ALU.add)
                        m_bc, rstd = ln_stats(y2, cs)
                        ln_apply(y2, xn2, cs, par, pb + 32, pb + 36, m_bc, rstd,
                                 skip_b[g][1])
                    x = xn2

                # output proj (pre-scaled by (1-rw)); host adds rw*tgt
                opw = sb.tile([128, KC * H], f32r, tag="io", bufs=1)
                nc.sync.dma_start(opw[:], dram[f"io{g}"].ap()[:, KC * H:2 * KC * H])
                for cs in tiles:
                    for m in range(KC):
                        ps = psp.tile([128, cs[1] - cs[0]], f32, tag="mm")
                        for k in range(KC):
                            nc.tensor.matmul(
                                ps[:],
                                opw[:, k * H + 128 * m:k * H + 128 * (m + 1)],
                                x[:, k, cs[0]:cs[1]],
                                start=(k == 0), stop=(k == KC - 1))
                        ot = sb.tile([128, cs[1] - cs[0]], f32, tag="o", bufs=1)
                        nc.vector.tensor_scalar(
                            ot[:], ps[:], par[:, _P_OPB + m:_P_OPB + m + 1],
                            None, ALU.add)
                        nc.sync.dma_start(
                            dram[f"out{g}"].ap()[:, m * C + cs[0]:m * C + cs[1]], ot[:])

    nc.compile()
    return nc


_CACHE = {}


def _get_program(C0, C1, skip_b):
    key = (C0, C1, skip_b)
    if key not in _CACHE:
        _CACHE[key] = _build_program(C0, C1, skip_b)
    return _CACHE[key]


def _prep_gen_weights(i, g_ipw, g_ipb, g_qkv_w, g_qkv_b, g_ao_w, g_ao_b,
                      g_ln1g, g_ln1b, g_ln2g, g_ln2b, g_f1w, g_f1b, g_f2w,
                      g_f2b, g_opw, g_opb, g_rw):
    wa, ba = [], []
    for l in range(L):
        _wq, _wk, wv = np.split(g_qkv_w[i, l], 3, axis=0)
        _bq, _bk, bv = np.split(g_qkv_b[i, l], 3)
        wa.append((g_ao_w[i, l] @ wv).T)                 # [K=H, M=H]
        ba.append(g_ao_b[i, l] + bv @ g_ao_w[i, l].T)
    rw = float(g_rw[i])
    io = np.concatenate([_sb_pack(_round_f32r(g_ipw[i].T)),
                         _sb_pack(_round_f32r((1.0 - rw) * g_opw[i].T))], axis=1)
    waP = np.stack([_sb_pack(_round_f32r(wa[l])) for l in range(L)])
    f1P = np.stack([_sb_pack(_round_f32r(g_f1w[i, l].T)) for l in range(L)])
    f2P = np.stack([_sb_pack(_round_f32r(g_f2w[i, l].T)) for l in range(L)])

    par = np.zeros((128, 128), np.float32)
    par[:, _P_IPB:_P_IPB + KC] = _pack_pcol(g_ipb[i])
    for l in range(L):
        pb = _P_LAYER + 40 * l
        par[:, pb:pb + 4] = _pack_pcol(ba[l])
        par[:, pb + 4:pb + 20] = _pack_pcol(g_f1b[i, l])
        par[:, pb + 20:pb + 24] = _pack_pcol(g_f2b[i, l])
        par[:, pb + 24:pb + 28] = _pack_pcol(g_ln1g[i, l])
        par[:, pb + 28:pb + 32] = _pack_pcol(g_ln1b[i, l])
        par[:, pb + 32:pb + 36] = _pack_pcol(g_ln2g[i, l])
        par[:, pb + 36:pb + 40] = _pack_pcol(g_ln2b[i, l])
    par[:, _P_OPB:_P_OPB + KC] = _pack_pcol((1.0 - rw) * g_opb[i])

    skip = (bool(np.all(g_ln1b[i] == 0.0)), bool(np.all(g_ln2b[i] == 0.0)))
    return {"io": io, "wa": waP, "f1": f1P, "f2": f2P, "par": par}, skip, rw


def _prepare(inputs):
    """Host-side prep. Returns (nc, in_maps, assemble) where assemble(results)
    builds the final outputs."""
    image = np.asarray(inputs["image_features"], np.float32)
    text = np.asarray(inputs["text_features"], np.float32)
    mt = np.asarray(inputs["missing_type"])

    idx1 = np.nonzero(mt == 1)[0]      # gen0 (img -> text) fills text
    idx2 = np.nonzero(mt == 2)[0]      # gen1 (text -> img) fills img
    idx3 = np.nonzero(mt == 3)[0]

    gw = {k: np.asarray(v) for k, v in inputs.items() if k.startswith("g_")}
    w0, skip0, rw0 = _prep_gen_weights(0, **gw)
    w1, skip1, rw1 = _prep_gen_weights(1, **gw)

    # prior MLP on host (tiny)
    pe = np.asarray(inputs["prior_emb"], np.float64)
    t = pe @ np.asarray(inputs["prior_w1"], np.float64).T + np.asarray(inputs["prior_b1"], np.float64)
    t = 0.5 * t * (1.0 + np.vectorize(math.erf)(t / math.sqrt(2.0)))
    prior = (t @ np.asarray(inputs["prior_w2"], np.float64).T
             + np.asarray(inputs["prior_b2"], np.float64)).astype(np.float32)
    p_img, p_text = prior[0, :H], prior[0, H:]

    imgT = np.ascontiguousarray(image.T)
    textT = np.ascontiguousarray(text.T)

    def shard_cols(Tsrc, Ttgt, idx):
        n_pc = max(1, -(-len(idx) // N_CORES))
        C = max(256, -(-n_pc // 64) * 64)
        pad = np.zeros(N_CORES * C, np.int64)
        pad[:len(idx)] = idx
        pad = pad.reshape(N_CORES, C)
        return C, [_sb_pack(_round_f32r(Tsrc[:, pad[c]])) for c in range(N_CORES)], \
            [_sb_pack(_round_f32r(Ttgt[:, pad[c]])) for c in range(N_CORES)]

    C0, src0, tgt0 = shard_cols(imgT, textT, idx1)
    C1, src1, tgt1 = shard_cols(textT, imgT, idx2)

    nc = _get_program(C0, C1, (skip0, skip1))

    ones = np.ones((128, 128), np.float32)
    in_maps = []
    for c in range(N_CORES):
        in_maps.append({
            "src0": src0[c], "tgt0": tgt0[c], "src1": src1[c], "tgt1": tgt1[c],
            "io0": w0["io"], "wa0": w0["wa"], "f10": w0["f1"], "f20": w0["f2"],
            "par0": w0["par"],
            "io1": w1["io"], "wa1": w1["wa"], "f11": w1["f1"], "f21": w1["f2"],
            "par1": w1["par"],
            "ones": ones,
        })

    def assemble(results):
        def gather_out(name, C, idx, rw, full):
            cols = [results[c][name].reshape(128, KC, C).transpose(1, 0, 2).reshape(H, C)
                    for c in range(N_CORES)]
            allc = np.concatenate(cols, axis=1)[:, :len(idx)]
            return rw * full[idx] + allc.T

        enhanced_text = text.copy()
        if len(idx1):
            enhanced_text[idx1] = gather_out("out0", C0, idx1, rw0, text)
        enhanced_img = image.copy()
        if len(idx2):
            enhanced_img[idx2] = gather_out("out1", C1, idx2, rw1, image)
        if len(idx3):
            enhanced_img[idx3] = p_img
            enhanced_text[idx3] = p_text
        return enhanced_img, enhanced_text

    return nc, in_maps, assemble


def kernel(**inputs):
    nc, in_maps, assemble = _prepare(inputs)
    res = run_bass_kernel_spmd(nc, in_maps, list(range(N_CORES)))
    return assemble(res.results)


# revision 7
# speedup vs baseline: 227.1638x; 7.2483x over previous
"""Trainium2 Bass kernel for nn_ModalGenerator (MoE-routed cross-modal generator).

Strategy:
  - seq_len==1 => softmax over one key is identically 1, so attention output is
    just v = tgt @ wv.T + bv. Fold wv/ao_w into one 512x512 matrix per layer
    (host-side), and fold (1-rw) into the output projection.
  - MoE routing: only columns with missing_type==1 need generator 0 (img->text)
    and only missing_type==2 need generator 1 (text->img). Host gathers those
    columns, the device runs the generators on the compacted columns only
    (~1/4 of the batch each), host scatters results back. missing_type==3 rows
    use the (host-computed, tiny) prior MLP; other rows pass through.
  - Data-parallel over 8 NeuronCores: columns sharded, weights replicated.
  - Activations live transposed [H(partitions), cols(free)]. All matmuls in
    float32r (e8m11) at full PE rate. LayerNorm stats via ones-matmul (column
    sums, already broadcast across partitions), rsqrt via bit-hack + Newton on
    VectorE, exact Gelu on ScalarE (single ACT table set for the whole kernel).
"""

import math

import numpy as np

import concourse.bacc as bacc
import concourse.mybir as mybir
import concourse.tile as tile
from concourse.bass_utils import run_bass_kernel_spmd

f32 = mybir.dt.float32
f32r = mybir.dt.float32r
i32 = mybir.dt.int32
AF = mybir.ActivationFunctionType
ALU = mybir.AluOpType

H = 512
L = 3
N_CORES = 8
KC = H // 128            # 4 K-chunks of the hidden dim
FH = 4 * H               # 2048 FFN hidden
FKC = FH // 128          # 16
LN_EPS = 1e-5
MAGIC = 0x5F3759DF

# param pack column layout (per generator): [128, 128] f32
_P_IPB = 0
_P_LAYER = 4             # + 40*l: ba 0..3 | f1b 4..19 | f2b 20..23
#                                 | ln1g 24..27 | ln1b 28..31 | ln2g 32..35 | ln2b 36..39
_P_OPB = 124


def _round_f32r(a):
    """Round-to-nearest-even fp32 -> fp32r (e8m11: low 12 mantissa bits zero)."""
    b = np.ascontiguousarray(a, dtype=np.float32).view(np.uint32).copy()
    b += np.uint32(0x7FF) + ((b >> np.uint32(12)) & np.uint32(1))
    b &= np.uint32(0xFFFFF000)
    return b.view(np.float32)


def _pack_pcol(vec):
    """[n*128] vector -> [128, n] chunk-column layout."""
    return np.ascontiguousarray(np.asarray(vec, np.float32).reshape(-1, 128).T)


def _sb_pack(wT):
    """[K, M] (K mult of 128) -> [128, (K/128)*M] SBUF chunk-major layout."""
    K, M = wT.shape
    return np.ascontiguousarray(
        wT.reshape(K // 128, 128, M).transpose(1, 0, 2).reshape(128, -1))


def _ntiles(C):
    if C <= 512:
        return [(0, C)]
    h = ((C // 2) + 31) // 32 * 32
    return [(0, h), (h, C)]


def _build_program(C0, C1, skip_b, repeat=1):
    nc = bacc.Bacc("TRN2", target_bir_lowering=False, debug=False,
                   num_devices=N_CORES)

    dram = {}
    for g, C in ((0, C0), (1, C1)):
        dram[f"src{g}"] = nc.dram_tensor(f"src{g}", [128, KC * C], f32r, kind="ExternalInput")
        dram[f"tgt{g}"] = nc.dram_tensor(f"tgt{g}", [128, KC * C], f32r, kind="ExternalInput")
        dram[f"io{g}"] = nc.dram_tensor(f"io{g}", [128, 2 * KC * H], f32r, kind="ExternalInput")
        dram[f"wa{g}"] = nc.dram_tensor(f"wa{g}", [L, 128, KC * H], f32r, kind="ExternalInput")
        dram[f"f1{g}"] = nc.dram_tensor(f"f1{g}", [L, 128, KC * FH], f32r, kind="ExternalInput")
        dram[f"f2{g}"] = nc.dram_tensor(f"f2{g}", [L, 128, FKC * H], f32r, kind="ExternalInput")
        dram[f"par{g}"] = nc.dram_tensor(f"par{g}", [128, 128], f32, kind="ExternalInput")
        dram[f"out{g}"] = nc.dram_tensor(f"out{g}", [128, KC * C], f32, kind="ExternalOutput")
    dram["ones"] = nc.dram_tensor("ones", [128, 128], f32r, kind="ExternalInput")

    h_bufs = 2 if max(C0, C1) <= 512 else 1

    with tile.TileContext(nc) as tc:
        with (
            tc.tile_pool(name="sb", bufs=2) as sb,
            tc.tile_pool(name="ps", bufs=4, space="PSUM") as psp,
        ):
            ones = sb.tile([128, 128], f32r, tag="ones", bufs=1)
            nc.sync.dma_start(ones[:], dram["ones"].ap())

            def ln_stats(y, cs):
                """column sums of y and y^2 (broadcast over partitions) -> rstd, m."""
                c0, c1 = cs
                Ct = c1 - c0
                s_ps = psp.tile([128, Ct], f32, tag="s", bufs=2)
                q_ps = psp.tile([128, Ct], f32, tag="q", bufs=2)
                for k in range(KC):
                    nc.tensor.matmul(s_ps[:], ones[:], y[:, k, c0:c1],
                                     start=(k == 0), stop=(k == KC - 1))
                for k in range(KC):
                    ysq = sb.tile([128, Ct], f32r, tag="ysq")
                    nc.scalar.activation(ysq[:], y[:, k, c0:c1], AF.Square)
                    nc.tensor.matmul(q_ps[:], ones[:], ysq[:],
                                     start=(k == 0), stop=(k == KC - 1))
                m_bc = sb.tile([128, Ct], f32, tag="m")
                nc.vector.tensor_scalar(m_bc[:], s_ps[:], 1.0 / H, None, ALU.mult)
                msq = sb.tile([128, Ct], f32, tag="msq", bufs=1)
                nc.vector.tensor_mul(msq[:], m_bc[:], m_bc[:])
                z = sb.tile([128, Ct], f32, tag="z")
                nc.vector.scalar_tensor_tensor(z[:], q_ps[:], 1.0 / H, msq[:],
                                               ALU.mult, ALU.subtract)
                nc.vector.tensor_scalar(z[:], z[:], LN_EPS, None, ALU.add)
                ti = sb.tile([128, Ct], i32, tag="ti", bufs=1)
                nc.vector.tensor_scalar(ti[:], z[:].bitcast(i32), 1, None,
                                        ALU.arith_shift_right)
                rstd = sb.tile([128, Ct], f32, tag="rstd", bufs=2)
                nc.vector.tensor_scalar(rstd[:].bitcast(i32), ti[:], -1, MAGIC,
                                        ALU.mult, ALU.add)
                for _ in range(2):
                    u = sb.tile([128, Ct], f32, tag="u", bufs=1)
                    nc.vector.tensor_mul(u[:], rstd[:], rstd[:])
                    w = sb.tile([128, Ct], f32, tag="w", bufs=1)
                    nc.vector.scalar_tensor_tensor(w[:], u[:], -0.5, z[:],
                                                   ALU.mult, ALU.mult)
                    rstd2 = sb.tile([128, Ct], f32, tag="rstd", bufs=2)
                    nc.vector.scalar_tensor_tensor(rstd2[:], w[:], 1.5, rstd[:],
                                                   ALU.add, ALU.mult)
                    rstd = rstd2
                return m_bc, rstd

            def ln_apply(y, xn, cs, par, gcol, bcol, m_bc, rstd, skip_beta):
                c0, c1 = cs
                for m in range(KC):
                    u1 = sb.tile([128, c1 - c0], f32, tag="u1")
                    nc.vector.tensor_sub(u1[:], y[:, m, c0:c1], m_bc[:])
                    if skip_beta:
                        nc.vector.scalar_tensor_tensor(
                            xn[:, m, c0:c1], u1[:], par[:, gcol + m:gcol + m + 1],
                            rstd[:], ALU.mult, ALU.mult)
                    else:
                        u2 = sb.tile([128, c1 - c0], f32, tag="u2", bufs=1)
                        nc.vector.scalar_tensor_tensor(
                            u2[:], u1[:], par[:, gcol + m:gcol + m + 1],
                            rstd[:], ALU.mult, ALU.mult)
                        nc.vector.tensor_scalar(
                            xn[:, m, c0:c1], u2[:], par[:, bcol + m:bcol + m + 1],
                            None, ALU.add)

            for _rep in range(repeat):
             for g, C in ((0, C0), (1, C1)):
                tiles = _ntiles(C)
                src = sb.tile([128, KC, C], f32r, tag="x", bufs=2)
                tgt = sb.tile([128, KC, C], f32r, tag="tgt", bufs=1)
                nc.sync.dma_start(src[:], dram[f"src{g}"].ap())
                nc.sync.dma_start(tgt[:], dram[f"tgt{g}"].ap())
                par = sb.tile([128, 128], f32, tag="par", bufs=1)
                nc.sync.dma_start(par[:], dram[f"par{g}"].ap())
                iow = sb.tile([128, KC * H], f32r, tag="io", bufs=1)
                nc.sync.dma_start(iow[:], dram[f"io{g}"].ap()[:, 0:KC * H])

                # input proj: x = ipwT.T @ src + ipb
                x = sb.tile([128, KC, C], f32r, tag="x", bufs=2)
                for cs in tiles:
                    for m in range(KC):
                        ps = psp.tile([128, cs[1] - cs[0]], f32, tag="mm")
                        for k in range(KC):
                            nc.tensor.matmul(
                                ps[:], iow[:, k * H + 128 * m:k * H + 128 * (m + 1)],
                                src[:, k, cs[0]:cs[1]],
                                start=(k == 0), stop=(k == KC - 1))
                        nc.vector.tensor_scalar(
                            x[:, m, cs[0]:cs[1]], ps[:],
                            par[:, _P_IPB + m:_P_IPB + m + 1], None, ALU.add)

                for l in range(L):
                    pb = _P_LAYER + 40 * l
                    wa = sb.tile([128, KC * H], f32r, tag="wa", bufs=2)
                    nc.sync.dma_start(wa[:], dram[f"wa{g}"].ap()[l])
                    f1w = sb.tile([128, KC * FH], f32r, tag="f1", bufs=1)
                    nc.sync.dma_start(f1w[:], dram[f"f1{g}"].ap()[l])
                    f2w = sb.tile([128, FKC, H], f32r, tag="f2", bufs=1)
                    nc.sync.dma_start(f2w[:], dram[f"f2{g}"].ap()[l])

                    # ---- attention(=v proj) + residual + LN1 ----
                    xn = sb.tile([128, KC, C], f32r, tag="x", bufs=2)
                    for cs in tiles:
                        y = sb.tile([128, KC, C], f32r, tag="y")
                        for m in range(KC):
                            ps = psp.tile([128, cs[1] - cs[0]], f32, tag="mm")
                            for k in range(KC):
                                nc.tensor.matmul(
                                    ps[:], wa[:, k * H + 128 * m:k * H + 128 * (m + 1)],
                                    tgt[:, k, cs[0]:cs[1]],
                                    start=(k == 0), stop=(k == KC - 1))
                            nc.vector.scalar_tensor_tensor(
                                y[:, m, cs[0]:cs[1]], ps[:],
                                par[:, pb + m:pb + m + 1], x[:, m, cs[0]:cs[1]],
                                ALU.add, ALU.add)
                        m_bc, rstd = ln_stats(y, cs)
                        ln_apply(y, xn, cs, par, pb + 24, pb + 28, m_bc, rstd,
                                 skip_b[g][0])
                    x = xn

                    # ---- FFN + residual + LN2 ----
                    xn2 = sb.tile([128, KC, C], f32r, tag="x", bufs=2)
                    for cs in tiles:
                        Ct = cs[1] - cs[0]
                        hh = sb.tile([128, FKC, Ct], f32r, tag="h", bufs=h_bufs)
                        for m in range(FKC):
                            ps = psp.tile([128, Ct], f32, tag="mm")
                            for k in range(KC):
                                nc.tensor.matmul(
                                    ps[:], f1w[:, k * FH + 128 * m:k * FH + 128 * (m + 1)],
                                    x[:, k, cs[0]:cs[1]],
                                    start=(k == 0), stop=(k == KC - 1))
                            nc.scalar.activation(hh[:, m, :], ps[:], AF.Gelu,
                                                 bias=par[:, pb + 4 + m:pb + 4 + m + 1])
                        y2 = sb.tile([128, KC, C], f32r, tag="y")
                        for m in range(KC):
                            ps = psp.tile([128, Ct], f32, tag="mm")
                            for k in range(FKC):
                                nc.tensor.matmul(
                                    ps[:], f2w[:, k, 128 * m:128 * (m + 1)], hh[:, k, :],
                                    start=(k == 0), stop=(k == FKC - 1))
                            nc.vector.scalar_tensor_tensor(
                                y2[:, m, cs[0]:cs[1]], ps[:],
                                par[:, pb + 20 + m:pb + 20 + m + 1],
                                x[:, m, cs[0]:cs[1]], ALU.add, ALU.add)
                        m_bc, rstd = ln_stats(y2, cs)
                        ln_apply(y2, xn2, cs, par, pb + 32, pb + 36, m_bc, rstd,
                                 skip_b[g][1])
                    x = xn2

                # output proj (pre-scaled by (1-rw)); host adds rw*tgt
                opw = sb.tile([128, KC * H], f32r, tag="io", bufs=1)
                nc.sync.dma_start(opw[:], dram[f"io{g}"].ap()[:, KC * H:2 * KC * H])
                for cs in tiles:
                    for m in range(KC):
                        ps = psp.tile([128, cs[1] - cs[0]], f32, tag="mm")
                        for k in range(KC):
                            nc.tensor.matmul(
                                ps[:],
                                opw[:, k * H + 128 * m:k * H + 128 * (m + 1)],
                                x[:, k, cs[0]:cs[1]],
                                start=(k == 0), stop=(k == KC - 1))
                        ot = sb.tile([128, cs[1] - cs[0]], f32, tag="o", bufs=1)
                        nc.vector.tensor_scalar(
                            ot[:], ps[:], par[:, _P_OPB + m:_P_OPB + m + 1],
                            None, ALU.add)
                        nc.sync.dma_start(
                            dram[f"out{g}"].ap()[:, m * C + cs[0]:m * C + cs[1]], ot[:])

    nc.compile()
    return nc


_CACHE = {}


def _get_program(C0, C1, skip_b, repeat=1):
    key = (C0, C1, skip_b, repeat)
    if key not in _CACHE:
        _CACHE[key] = _build_program(C0, C1, skip_b, repeat)
    return _CACHE[key]


def _prep_gen_weights(i, g_ipw, g_ipb, g_qkv_w, g_qkv_b, g_ao_w, g_ao_b,
                      g_ln1g, g_ln1b, g_ln2g, g_ln2b, g_f1w, g_f1b, g_f2w,
                      g_f2b, g_opw, g_opb, g_rw):
    wa, ba = [], []
    for l in range(L):
        _wq, _wk, wv = np.split(g_qkv_w[i, l], 3, axis=0)
        _bq, _bk, bv = np.split(g_qkv_b[i, l], 3)
        wa.append((g_ao_w[i, l] @ wv).T)                 # [K=H, M=H]
        ba.append(g_ao_b[i, l] + bv @ g_ao_w[i, l].T)
    rw = float(g_rw[i])
    io = np.concatenate([_sb_pack(_round_f32r(g_ipw[i].T)),
                         _sb_pack(_round_f32r((1.0 - rw) * g_opw[i].T))], axis=1)
    waP = np.stack([_sb_pack(_round_f32r(wa[l])) for l in range(L)])
    f1P = np.stack([_sb_pack(_round_f32r(g_f1w[i, l].T)) for l in range(L)])
    f2P = np.stack([_sb_pack(_round_f32r(g_f2w[i, l].T)) for l in range(L)])

    par = np.zeros((128, 128), np.float32)
    par[:, _P_IPB:_P_IPB + KC] = _pack_pcol(g_ipb[i])
    for l in range(L):
        pb = _P_LAYER + 40 * l
        par[:, pb:pb + 4] = _pack_pcol(ba[l])
        par[:, pb + 4:pb + 20] = _pack_pcol(g_f1b[i, l])
        par[:, pb + 20:pb + 24] = _pack_pcol(g_f2b[i, l])
        par[:, pb + 24:pb + 28] = _pack_pcol(g_ln1g[i, l])
        par[:, pb + 28:pb + 32] = _pack_pcol(g_ln1b[i, l])
        par[:, pb + 32:pb + 36] = _pack_pcol(g_ln2g[i, l])
        par[:, pb + 36:pb + 40] = _pack_pcol(g_ln2b[i, l])
    par[:, _P_OPB:_P_OPB + KC] = _pack_pcol((1.0 - rw) * g_opb[i])

    skip = (bool(np.all(g_ln1b[i] == 0.0)), bool(np.all(g_ln2b[i] == 0.0)))
    return {"io": io, "wa": waP, "f1": f1P, "f2": f2P, "par": par}, skip, rw


def _prepare(inputs, repeat=1):
    """Host-side prep. Returns (nc, in_maps, assemble) where assemble(results)
    builds the final outputs."""
    image = np.asarray(inputs["image_features"], np.float32)
    text = np.asarray(inputs["text_features"], np.float32)
    mt = np.asarray(inputs["missing_type"])

    idx1 = np.nonzero(mt == 1)[0]      # gen0 (img -> text) fills text
    idx2 = np.nonzero(mt == 2)[0]      # gen1 (text -> img) fills img
    idx3 = np.nonzero(mt == 3)[0]

    gw = {k: np.asarray(v) for k, v in inputs.items() if k.startswith("g_")}
    w0, skip0, rw0 = _prep_gen_weights(0, **gw)
    w1, skip1, rw1 = _prep_gen_weights(1, **gw)

    # prior MLP on host (tiny)
    pe = np.asarray(inputs["prior_emb"], np.float64)
    t = pe @ np.asarray(inputs["prior_w1"], np.float64).T + np.asarray(inputs["prior_b1"], np.float64)
    t = 0.5 * t * (1.0 + np.vectorize(math.erf)(t / math.sqrt(2.0)))
    prior = (t @ np.asarray(inputs["prior_w2"], np.float64).T
             + np.asarray(inputs["prior_b2"], np.float64)).astype(np.float32)
    p_img, p_text = prior[0, :H], prior[0, H:]

    imgT = np.ascontiguousarray(image.T)
    textT = np.ascontiguousarray(text.T)

    def shard_cols(Tsrc, Ttgt, idx):
        n_pc = max(1, -(-len(idx) // N_CORES))
        C = max(256, -(-n_pc // 64) * 64)
        pad = np.zeros(N_CORES * C, np.int64)
        pad[:len(idx)] = idx
        pad = pad.reshape(N_CORES, C)
        return C, [_sb_pack(_round_f32r(Tsrc[:, pad[c]])) for c in range(N_CORES)], \
            [_sb_pack(_round_f32r(Ttgt[:, pad[c]])) for c in range(N_CORES)]

    C0, src0, tgt0 = shard_cols(imgT, textT, idx1)
    C1, src1, tgt1 = shard_cols(textT, imgT, idx2)

    nc = _get_program(C0, C1, (skip0, skip1), repeat)

    ones = np.ones((128, 128), np.float32)
    in_maps = []
    for c in range(N_CORES):
        in_maps.append({
            "src0": src0[c], "tgt0": tgt0[c], "src1": src1[c], "tgt1": tgt1[c],
            "io0": w0["io"], "wa0": w0["wa"], "f10": w0["f1"], "f20": w0["f2"],
            "par0": w0["par"],
            "io1": w1["io"], "wa1": w1["wa"], "f11": w1["f1"], "f21": w1["f2"],
            "par1": w1["par"],
            "ones": ones,
        })

    def assemble(results):
        def gather_out(name, C, idx, rw, full):
            cols = [results[c][name].reshape(128, KC, C).transpose(1, 0, 2).reshape(H, C)
                    for c in range(N_CORES)]
            allc = np.concatenate(cols, axis=1)[:, :len(idx)]
            return rw * full[idx] + allc.T

        enhanced_text = text.copy()
        if len(idx1):
            enhanced_text[idx1] = gather_out("out0", C0, idx1, rw0, text)
        enhanced_img = image.copy()
        if len(idx2):
            enhanced_img[idx2] = gather_out("out1", C1, idx2, rw1, image)
        if len(idx3):
            enhanced_img[idx3] = p_img
            enhanced_text[idx3] = p_text
        return enhanced_img, enhanced_text

    return nc, in_maps, assemble


def kernel(**inputs):
    nc, in_maps, assemble = _prepare(inputs)
    res = run_bass_kernel_spmd(nc, in_maps, list(range(N_CORES)))
    return assemble(res.results)
